# revision 95
# baseline (speedup 1.0000x reference)
"""Trainium2 Bass kernel for nn_BHSDuelingDQN (gnn_message_passing).

Math notes (validated vs reference to fp32 precision):
  - The edge MLP input is ones(E,1), so every edge shares one theta [F,OUT]:
        theta = (relu(w1[0]+b1) @ w2 + b2).reshape(F, OUT)
  - edge_index values live in [0, N), so the gather/scatter-add only touches
    batch 0 of flat=[B*N,F].  With C[s,t] = #edges(src=s, tgt=t):
        agg(batch0) = C^T @ (x[0] @ theta)
    which turns the whole message passing into dense matmuls.  C is built on
    the host from edge_index (pure index bookkeeping; all FLOPs with x /
    theta / weights stay on device).

Sharding: phase 1 is node-sharded (each of 8 cores owns 32 of 256 nodes and
computes partial pre-activations of adv/v1 for all 1024 batches over its
4096 feature rows).  Phase 2 is batch-sharded (each core sums the 8 partials
for its 128 batches and runs the small value-head + dueling combine).  The
host only slices / concatenates / transposes arrays between phases.

Phase 1 (PE-roofline bound: conv 32768 + accumulate 65536 bf16 columns):
  - x is stored compactly in bf16 as [8, node, batch] with root_w riding at
    the head of the same tensor, so every conv matmul contracts partitions
    0..8 with a shared stationary root weight (fp8 was evaluated for the
    accumulate matmuls -- DoubleRow would halve the PE cycles -- but e4m3's
    1.8% element noise puts the output at 2.8e-2 rel err vs the 2e-2 gate
    because the dueling combine cancels signal but not noise).
  - one flat conv/acc pipeline across four batch sweeps (512/256/192/64 --
    the last narrow so the final drain+DMA tail is short); convs run 5 (wide
    sweep) / 9 (narrow sweeps, where two nodes share one PSUM bank and a
    single pair relu) nodes ahead of the accumulates; relu alternates
    Act/DVE (GPSIMD cannot read PSUM).
  - warm-up matmuls bridge the initial DMA latency and the p-state ramp.
  - DMAs are deadline-ordered: a tiny hot tensor (conv bias / head bias
    rows / edge-MLP params), then x and head-weight chunks interleaved.
Phase 2 runs the 8-partial reduction ON THE PE: partials are packed 4 per
round x 32 rows on partitions and contracted against a stacked-identity
selector (4 x 32x32 eyes, built on device via affine_select so no DMA slot
is spent on it), accumulating both rounds into PSUM; warm-up matmuls bridge
the p-state ramp while the partial halves stream in.  That
replaces the f16 DVE adder tree (faster AND more accurate: f32 PSUM).  The
dueling mean/expand structure is folded into the weights on the host (pure
index structure); biases ride as activation bias columns.

HW-ISA notes learned the hard way: GPSIMD cannot access PSUM; fp32r matmuls
need moving-dim >= 256 for full rate and reject 1-column outputs; matmul
fmap/weight must share their SBUF start partition, which must equal the PE
row tile position; f32->f32r bitcasts of engine outputs are rejected (only
engines and DMA may produce f32r); f32r/f16 memsets are invalid ISA.
"""

import os
from contextlib import ExitStack

import numpy as np

import concourse.bacc as bacc
import concourse.bass as bass
import concourse.mybir as mybir
import concourse.tile as tile
from concourse import masks
from concourse.bass_utils import run_bass_kernel_spmd  # noqa: F401  (contract)

F32 = mybir.dt.float32
F16 = mybir.dt.float16
BF16 = mybir.dt.bfloat16

B, N, F, E, OUT, NDIV, PER = 1024, 256, 8, 1024, 128, 64, 3
NADV = NDIV * PER            # 192
AV = NADV + 64               # 256 fused output cols of phase 1 (adv | v1)
M = 8                        # cores
NPC = N // M                 # 32 nodes per core
MODE = os.environ.get("BASS_KERNEL_MODE", "f32r")  # kept for test.py compat

WARM_N = int(os.environ.get("BASS_WARM_N", "3"))
WARM2_N = int(os.environ.get("BASS_WARM2_N", "5"))

# hot tensor (f32, first DMA): per-partition columns
HOT_CB, HOT_BR0, HOT_BR1, HOT_W1, HOT_B1 = 0, 1, 2, 3, 4
HOT_COLS = 13                 # cols 5..13 reserved

# xb tensor (bf16, 8 partitions): root_w + x[0] local nodes + x stream
XRW = 0                       # [8, 128] root_w
XX0 = 128                     # [8, 32] x[0] local nodes transposed
XND = 160                     # 32 nodes x 1024 batch-permuted columns
XB_COLS = XND + NPC * B

# pa0 column map (f32, scalar queue, phase-0 chain only)
PA_B2T = 0                    # 8 cols  [128, 8] = b2.reshape(F, OUT).T
PA_X0T = 8                    # 256 cols [8, 256] = x[0].T
PA_C = 264                    # 64 cols  [128, 2, 32] edge-count matrix
PA0_COLS = PA_C + 2 * NPC     # 328

# batch sweeps: (xb column base, width, output batch base); later sweeps
# narrow so the final drain + output DMA tail is short.  Batch 0 (the only
# one with a scatter-add contribution) sits at sweep 1's first column.
SWEEPS = ((0, 512, 512), (512, 256, 0), (768, 192, 256), (960, 64, 448))

_build_cache = {}


def _build_phase1(repeat=1):
    nc = bacc.Bacc("TRN2")

    hot_d = nc.dram_tensor("hot", [128, HOT_COLS], F32, kind="ExternalInput")
    xb_d = nc.dram_tensor("xb", [F, XB_COLS], BF16, kind="ExternalInput")
    wb_d = nc.dram_tensor("wb", [128, 8 * 1024], BF16, kind="ExternalInput")
    pa0_d = nc.dram_tensor("pa0", [128, PA0_COLS], F32, kind="ExternalInput")
    pw2_d = nc.dram_tensor("pw2", [64, F * OUT], BF16, kind="ExternalInput")
    pt_d = nc.dram_tensor("pt", [128, 2, B], BF16, kind="ExternalOutput")

    with tile.TileContext(nc) as tc:
      for rep in range(repeat):
        with ExitStack() as ctx:
            const = ctx.enter_context(tc.tile_pool(name=f"const{rep}", bufs=1))

            # tiny device-generated tensors first: no DMA deps
            wsmall_sb = const.tile([128, 16], F32, name="wsmall_sb")
            nc.gpsimd.memset(wsmall_sb, 0.0)
            ident_sb = const.tile([128, 128], F32, name="ident_sb")
            masks.make_identity(nc, ident_sb)

            # DMAs in deadline order (HWDGE desc-gen is a shared ~625ns/DMA
            # resource; DMA_ENGINES transfers serialize at ~360GB/s)
            hot_sb = const.tile([128, HOT_COLS], F32, name="hot_sb")
            xb_sb = const.tile([F, XB_COLS], BF16, name="xb_sb")
            wb_sb = const.tile([128, 8 * 1024], BF16, name="wb_sb")
            pa0_sb = const.tile([128, PA0_COLS], F32, name="pa0_sb")
            pw2_sb = const.tile([64, F * OUT], BF16, name="pw2_sb")
            xe = XND + 8 * 1024

            # all input DMAs ride ONE queue in deadline order (a second
            # queue's dma_starts would interleave into the shared HWDGE's
            # ~625ns/DMA desc-gen slots and delay the early chunks)
            nc.sync.dma_start(out=xb_sb[:, 0:xe], in_=xb_d[:, 0:xe])
            nc.sync.dma_start(out=hot_sb, in_=hot_d[:])
            nc.sync.dma_start(out=wb_sb[:, 0:1024], in_=wb_d[:, 0:1024])
            nc.sync.dma_start(out=wb_sb[:, 1024:2048], in_=wb_d[:, 1024:2048])
            nc.sync.dma_start(out=xb_sb[:, xe : xe + 8192], in_=xb_d[:, xe : xe + 8192])
            nc.sync.dma_start(out=wb_sb[:, 2048:4096], in_=wb_d[:, 2048:4096])
            nc.sync.dma_start(out=xb_sb[:, xe + 8192 : xe + 16384], in_=xb_d[:, xe + 8192 : xe + 16384])
            nc.sync.dma_start(out=pa0_sb, in_=pa0_d[:])
            nc.sync.dma_start(out=pw2_sb, in_=pw2_d[:])
            nc.sync.dma_start(out=wb_sb[:, 4096:6144], in_=wb_d[:, 4096:6144])
            nc.sync.dma_start(out=xb_sb[:, xe + 16384 : xe + 24576], in_=xb_d[:, xe + 16384 : xe + 24576])
            nc.sync.dma_start(out=wb_sb[:, 6144:8192], in_=wb_d[:, 6144:8192])

            def pa(col, ncols, nrows=128):
                return pa0_sb[0:nrows, col : col + ncols]

            cb_ap = hot_sb[:, HOT_CB : HOT_CB + 1]
            rw_ap = xb_sb[0:F, XRW : XRW + OUT]

            # small SBUF intermediates
            h_sb = const.tile([64, 1], BF16, name="h_sb")
            thT_sb = const.tile([128, F], F32, name="thT_sb")
            th_sb = const.tile([F, OUT], F32, name="th_sb")
            x0th_sb = const.tile([128, 2, OUT], F32, name="x0th_sb")
            feat0_sb = const.tile([128, NPC], BF16, name="feat0_sb")

            acc_pool = ctx.enter_context(
                tc.tile_pool(name=f"accp{rep}", bufs=1, space="PSUM")
            )
            conv_pool = ctx.enter_context(
                tc.tile_pool(name=f"convp{rep}", bufs=5, space="PSUM")
            )
            p0_pool = ctx.enter_context(
                tc.tile_pool(name=f"p0p{rep}", bufs=1, space="PSUM")
            )
            feat_pool = ctx.enter_context(tc.tile_pool(name=f"featp{rep}", bufs=12))
            out_pool = ctx.enter_context(tc.tile_pool(name=f"outp{rep}", bufs=1))
            out_sb = out_pool.tile([128, 2, B], BF16, name="out_sb")

            # ---- PE warm-up: no-dep fp32 matmuls (4 cycles/row) bridge the
            # initial DMA latency and the p-state ramp (PE is 2-4x slower for
            # its first ~3us of busy time); narrow first so the PE starts the
            # moment the 16-col memset lands
            for k in range(5):
                warm_ps = conv_pool.tile(
                    [16, 16], F32, name="warm_ps", tag="conv_ps"
                )
                nc.tensor.matmul(warm_ps, wsmall_sb[:, 0:16], wsmall_sb)
            for k in range(WARM_N):
                warm_ps = conv_pool.tile(
                    [128, 128], F32, name="warm_ps", tag="conv_ps"
                )
                nc.tensor.matmul(warm_ps, ident_sb, ident_sb)

            def wc_ap(n, m):
                base = (n // 4) * 1024 + (n % 4) * 256 + m * 128
                return wb_sb[:, base : base + 128]

            RELU_ENG = ("act", "dve")

            def relu_op(eng, feat_ap, conv_ap):
                if eng == "act":
                    nc.scalar.activation(
                        feat_ap,
                        conv_ap,
                        mybir.ActivationFunctionType.Relu,
                        bias=cb_ap,
                    )
                else:
                    nc.vector.tensor_scalar(
                        feat_ap, conv_ap, cb_ap, 0.0,
                        mybir.AluOpType.add, mybir.AluOpType.max,
                    )

            def patch_op(eng, feat_col, n):
                # batch 0 gets the precomputed scatter-add corrected column
                if eng == "dve":
                    nc.vector.tensor_copy(feat_col, feat0_sb[:, n : n + 1])
                else:
                    nc.scalar.activation(
                        feat_col,
                        feat0_sb[:, n : n + 1],
                        mybir.ActivationFunctionType.Copy,
                    )

            def xnb(n, xbase, width):
                base = XND + n * B + xbase
                return xb_sb[0:F, base : base + width]

            def emit_conv(n, xbase, width, agg):
                conv_ps = conv_pool.tile(
                    [128, width], F32, name="conv_ps", tag="conv_ps"
                )
                nc.tensor.matmul(conv_ps, rw_ap, xnb(n, xbase, width))
                feat_sb = feat_pool.tile([128, width], BF16, name="feat_sb")
                eng = RELU_ENG[n % 2]
                relu_op(eng, feat_sb, conv_ps)
                if agg:
                    patch_op("dve" if n % 2 == 0 else "act",
                             feat_sb[:, 0:1], n)
                return feat_sb

            pair_ps_box = [None]

            def emit_conv_pair(n, xbase, width, agg):
                # narrow sweeps: two nodes share one PSUM bank so the relu
                # (alternating Act/DVE; GPSIMD cannot read PSUM) runs once
                # per pair, halving the per-op engine overhead
                if n % 2 == 0:
                    pair_ps = conv_pool.tile(
                        [128, 2, width], F32, name="pair_ps", tag="conv_ps"
                    )
                    nc.tensor.matmul(
                        pair_ps[:, 0, :], rw_ap, xnb(n, xbase, width),
                        start=True, stop=False, skip_group_check=True,
                    )
                    pair_ps_box[0] = pair_ps
                    return None
                pair_ps = pair_ps_box[0]
                nc.tensor.matmul(
                    pair_ps[:, 1, :], rw_ap, xnb(n, xbase, width),
                    start=False, stop=True, skip_group_check=True,
                )
                feat2 = feat_pool.tile([128, 2, width], BF16, name="feat2")
                eng = RELU_ENG[(n // 2) % 2]
                relu_op(eng, feat2, pair_ps)
                if agg:
                    patch_op("dve", feat2[:, 0, 0:1], n - 1)
                    patch_op("act", feat2[:, 1, 0:1], n)
                return feat2

            acc_tiles = {}

            def drain(si):
                xbase, width, bbase = SWEEPS[si]
                sl = slice(bbase, bbase + width)
                acc_ps = acc_tiles[si]
                nc.vector.tensor_scalar_add(
                    out_sb[:, 0, sl], acc_ps[0], hot_sb[:, HOT_BR0 : HOT_BR0 + 1]
                )
                nc.scalar.activation(
                    out_sb[:, 1, sl],
                    acc_ps[1],
                    mybir.ActivationFunctionType.Identity,
                    bias=hot_sb[:, HOT_BR1 : HOT_BR1 + 1],
                )
                nc.sync.dma_start(out=pt_d[:, :, sl], in_=out_sb[:, :, sl])

            def run_sweeps(inject=()):
                # one flat conv/acc pipeline across all sweeps: convs run a
                # few nodes ahead so sweep boundaries leave no PE gap
                inject = dict(inject)
                pend = []

                def emit_acc(si, n, feat_ap):
                    for m in range(2):
                        nc.tensor.matmul(
                            acc_tiles[si][m],
                            wc_ap(n, m),
                            feat_ap,
                            start=(n == 0),
                            stop=(n == NPC - 1),
                        )
                    if n == NPC - 1:
                        drain(si)

                events = [(si, n) for si in range(len(SWEEPS))
                          for n in range(NPC)]
                for gi, (si, n) in enumerate(events):
                    xbase, width, _ = SWEEPS[si]
                    if n == 0:
                        acc_tiles[si] = [
                            acc_pool.tile(
                                [128, width], F32,
                                name=f"acc{m}_{si}", tag=f"acc{m}",
                            )
                            for m in range(2)
                        ]
                    if width >= 512:
                        feat_sb = emit_conv(n, xbase, width, si == 1)
                        pend.append((si, n, feat_sb))
                    else:
                        feat2 = emit_conv_pair(n, xbase, width, si == 1)
                        if feat2 is not None:
                            pend.append((si, n - 1, feat2[:, 0, :]))
                            pend.append((si, n, feat2[:, 1, :]))
                    if gi in inject:
                        inject[gi]()
                    while len(pend) > (4 if width >= 512 else 8):
                        psi, pn, pf = pend.pop(0)
                        emit_acc(psi, pn, pf)
                for psi, pn, pf in pend:
                    emit_acc(psi, pn, pf)

            # ---- phase 0 steps: theta, x0@theta, aggT -------------------
            # interleaved into the h=1 sweep so the serial chain (with its
            # cross-engine semaphore latencies) hides behind conv/acc work
            def p0_theta():
                nc.vector.tensor_scalar(
                    h_sb, hot_sb[0:64, HOT_W1 : HOT_W1 + 1],
                    hot_sb[0:64, HOT_B1 : HOT_B1 + 1], 0.0,
                    mybir.AluOpType.add, mybir.AluOpType.max,
                )
                thT_ps = p0_pool.tile([128, F], F32, name="thT_ps", tag="p0")
                for f in range(F):
                    nc.tensor.matmul(
                        thT_ps[:, f : f + 1],
                        pw2_sb[:, f * OUT : (f + 1) * OUT],
                        h_sb,
                    )
                nc.vector.tensor_add(thT_sb, thT_ps, pa(PA_B2T, F))

            def p0_th():
                th_ps = p0_pool.tile([F, OUT], F32, name="th_ps", tag="p0")
                nc.tensor.transpose(th_ps, thT_sb[:, 0:F], ident_sb)
                nc.vector.tensor_copy(th_sb, th_ps)

            def p0_x0th(s):
                x0th_ps = p0_pool.tile(
                    [128, OUT], F32, name=f"x0th_ps{s}", tag="p0"
                )
                nc.tensor.matmul(
                    x0th_ps, pa(PA_X0T + s * 128, 128, F), th_sb
                )
                nc.vector.tensor_copy(x0th_sb[:, s, :], x0th_ps)

            def p0_feat0():
                agg_ps = p0_pool.tile([128, NPC], F32, name="agg_ps", tag="p0")
                for s in range(2):
                    nc.tensor.matmul(
                        agg_ps,
                        x0th_sb[:, s, :],
                        pa(PA_C + s * NPC, NPC),
                        start=(s == 0),
                        stop=False,
                    )
                nc.tensor.matmul(
                    agg_ps,
                    rw_ap,
                    xb_sb[0:F, XX0 : XX0 + NPC],
                    start=False,
                    stop=True,
                )
                nc.scalar.activation(
                    feat0_sb,
                    agg_ps,
                    mybir.ActivationFunctionType.Relu,
                    bias=cb_ap,
                )

            # sweep 0 carries the phase-0 chain; sweep 1 needs feat0 ready.
            # Injections are spaced so each step's cross-engine inputs are
            # ready before its PE op queues (the in-order PE queue would
            # head-of-line block the conv stream behind an unsatisfied wait)
            run_sweeps(inject={
                13: p0_theta,
                17: p0_th,
                22: lambda: p0_x0th(0),
                25: lambda: p0_x0th(1),
                28: p0_feat0,
            })

    nc.finalize()
    return nc


# phase-2 folded-weight layout (all structural matrices pre-multiplied into
# the weights on the host; the dueling mean/expand matrices are pure index
# structure so this is weight re-indexing, not model compute)
# phase-2 layout:
# the 4x32x32 stacked-eye partial selector is built ON DEVICE via
# affine_select (no DMA -- the tiny selector tensor's HWDGE slot was
# delaying the big partial transfers it gated)
# hb2 (f32 bias columns; engine scalar-operand APs must be fp32)
HB_V2B = 0      # [64, 1]    v2b at partitions 0:64
HB_CB0 = 1      # [128, 1]   v3b@EM0
HB_CB1 = 2      # [64, 1]    v3b@EM1
HB_COLS = 3
# pf (bf16): value-head weight first (earliest deadline), then the
# dueling-combine blocks
PF_V2W = 0      # [64, 64]   v2w at partitions 64:128
PF_B0 = 64      # [64, 128]  MGB@EM0
PF_A0 = 192     # [128, 128] I + MGA@EM0
PF_C0 = 320     # [64, 128]  V3@EM0
PF_B1 = 448     # [64, 64]   I + MGB@EM1
PF_A1 = 512     # [128, 64]  MGA@EM1
PF_C1 = 576     # [64, 64]   V3@EM1
PF_COLS = 640
BT = B // M      # 128 batches per core


def _build_phase2(repeat=1):
    nc = bacc.Bacc("TRN2")

    hb2_d = nc.dram_tensor("hb2", [128, HB_COLS], F32, kind="ExternalInput")
    # partials packed for the PE reduction: partition p = 32*c' + r holds
    # partial 4*t+c', row 32*(chunk)+r; half h=0 carries rows 128:256
    # (adv[128:192] | v1) so the serial value-head chain starts first
    pp_d = nc.dram_tensor("pp", [128, 2, 2, 4, BT], BF16, kind="ExternalInput")
    pf_d = nc.dram_tensor("pf", [128, PF_COLS], BF16, kind="ExternalInput")
    ot_d = nc.dram_tensor("ot", [128, 2, BT], F32, kind="ExternalOutput")

    with tile.TileContext(nc) as tc:
      for rep in range(repeat):
        with ExitStack() as ctx:
            const = ctx.enter_context(tc.tile_pool(name=f"c2_{rep}", bufs=1))

            # device-generated warm-up sources first: no DMA deps
            wsmall_sb = const.tile([128, 16], F32, name="wsmall_sb")
            nc.gpsimd.memset(wsmall_sb, 0.0)
            wz_sb = const.tile([128, 128], F32, name="wz_sb")
            nc.gpsimd.memset(wz_sb, 0.0)

            # stacked-eye selector built on device: sel[p, j] = (p%32 == j)
            sel_sb = const.tile([128, 32], BF16, name="sel_sb")
            nc.gpsimd.memset(sel_sb, 0.0)
            for k in range(4):
                nc.gpsimd.affine_select(
                    out=sel_sb, in_=sel_sb,
                    compare_op=mybir.AluOpType.not_equal, fill=1.0,
                    base=-32 * k, pattern=[[-1, 32]], channel_multiplier=1,
                )

            hb2_sb = const.tile([128, HB_COLS], F32, name="hb2_sb")
            pp_sb = const.tile([128, 2, 2, 4, BT], BF16, name="pp_sb")
            pf_sb = const.tile([128, PF_COLS], BF16, name="pf_sb")
            # one queue, deadline order (shared HWDGE desc-gen serializes);
            # pp split by half so the psumB reduction overlaps the second
            # half's transfer and uses the mid-clock ramp window
            nc.sync.dma_start(out=pp_sb[:, 0], in_=pp_d[:, 0])
            nc.sync.dma_start(out=pp_sb[:, 1], in_=pp_d[:, 1])
            nc.sync.dma_start(out=pf_sb, in_=pf_d[:])
            nc.sync.dma_start(out=hb2_sb, in_=hb2_d[:])

            ot_sb = const.tile([128, 2, BT], F32, name="ot_sb")
            nc.gpsimd.memset(ot_sb[64:128, 1, :], 0.0)

            work = ctx.enter_context(tc.tile_pool(name=f"work{rep}", bufs=1))
            psum = ctx.enter_context(
                tc.tile_pool(name=f"psum{rep}", bufs=1, space="PSUM")
            )
            warm_pool = ctx.enter_context(
                tc.tile_pool(name=f"warm{rep}", bufs=2, space="PSUM")
            )

            # PE warm-up while the partial DMA is in flight: without ~3us of
            # continuous PE busy the whole phase runs at the 2-4x slower
            # cold p-states
            for k in range(5):
                warm_ps = warm_pool.tile([16, 16], F32, name="warm_ps",
                                         tag="warm")
                nc.tensor.matmul(warm_ps, wsmall_sb[:, 0:16], wsmall_sb)
            for k in range(WARM2_N):
                warm_ps = warm_pool.tile([128, 128], F32, name="warm_ps",
                                         tag="warm")
                nc.tensor.matmul(warm_ps, wz_sb, wz_sb)

            sel = sel_sb[:]
            # 8-partial reduction on the PE: for each 32-row region,
            # accumulate round 0 (partials 0-3) and round 1 (partials 4-7)
            pB = psum.tile([128, BT], F32, name="pB", tag="pB")
            pA = psum.tile([128, BT], F32, name="pA", tag="pA")
            for h, p in ((0, pB), (1, pA)):
                for k4 in range(4):
                    for t in range(2):
                        nc.tensor.matmul(
                            p[32 * k4 : 32 * k4 + 32, :],
                            sel,
                            pp_sb[:, h, t, k4, :],
                            start=(t == 0),
                            stop=(t == 1),
                            tile_position=(0, 32 * k4),
                        )
                if h == 0:
                    # rows 128:256: adv[128:192] at partitions 0:64, v1 at
                    # 64:128 -- one relu covers ar1 and v1r
                    tB = work.tile([128, BT], BF16, name="tB")
                    nc.scalar.activation(
                        tB, pB, mybir.ActivationFunctionType.Relu
                    )
            tA = work.tile([128, BT], BF16, name="tA")
            nc.vector.tensor_scalar_max(tA, pA, 0.0)

            # v2 = relu(v1 @ v2w + v2b)
            v2_ps = psum.tile([64, BT], F32, name="v2_ps", tag="v2")
            nc.tensor.matmul(
                v2_ps, pf_sb[64:128, PF_V2W : PF_V2W + 64], tB[64:128, :]
            )
            v2r = work.tile([64, BT], BF16, name="v2r")
            nc.vector.tensor_scalar(
                v2r, v2_ps, hb2_sb[0:64, HB_V2B : HB_V2B + 1], 0.0,
                mybir.AluOpType.add, mybir.AluOpType.max,
            )

            # folded dueling combine (C-terms last: v2r lands latest)
            o0_ps = psum.tile([128, BT], F32, name="o0_ps", tag="o0")
            nc.tensor.matmul(o0_ps, pf_sb[0:64, PF_B0 : PF_B0 + 128],
                             tB[0:64, :], start=True, stop=False)
            nc.tensor.matmul(o0_ps, pf_sb[:, PF_A0 : PF_A0 + 128],
                             tA, start=False, stop=False)
            nc.tensor.matmul(o0_ps, pf_sb[0:64, PF_C0 : PF_C0 + 128],
                             v2r, start=False, stop=True)
            o1_ps = psum.tile([64, BT], F32, name="o1_ps", tag="o1")
            nc.tensor.matmul(o1_ps, pf_sb[0:64, PF_B1 : PF_B1 + 64],
                             tB[0:64, :], start=True, stop=False)
            nc.tensor.matmul(o1_ps, pf_sb[:, PF_A1 : PF_A1 + 64],
                             tA, start=False, stop=False)
            nc.tensor.matmul(o1_ps, pf_sb[0:64, PF_C1 : PF_C1 + 64],
                             v2r, start=False, stop=True)

            nc.scalar.activation(
                ot_sb[:, 0, :], o0_ps, mybir.ActivationFunctionType.Identity,
                bias=hb2_sb[:, HB_CB0 : HB_CB0 + 1],
            )
            nc.vector.tensor_scalar_add(
                ot_sb[0:64, 1, :], o1_ps, hb2_sb[0:64, HB_CB1 : HB_CB1 + 1]
            )
            nc.sync.dma_start(out=ot_d[:], in_=ot_sb[:])

    nc.finalize()
    return nc


def _get_programs(mode=None, repeat=1):
    key = repeat
    if key not in _build_cache:
        _build_cache[key] = (_build_phase1(repeat), _build_phase2(repeat))
    return _build_cache[key]


# batch permutation: xb column j holds batch PERM[j] (sweep layout)
PERM = np.concatenate([np.arange(512, 1024), np.arange(0, 512)])


def _prep_phase1_inputs(inputs, mode=None):
    import ml_dtypes

    x = np.ascontiguousarray(np.asarray(inputs["x"], np.float32))
    ei = np.asarray(inputs["edge_index"]).astype(np.int64)
    w1 = np.asarray(inputs["w1"], np.float32)
    b1 = np.asarray(inputs["b1"], np.float32)
    w2 = np.asarray(inputs["w2"], np.float32)
    b2 = np.asarray(inputs["b2"], np.float32)
    root_w = np.asarray(inputs["root_w"], np.float32)
    conv_b = np.asarray(inputs["conv_b"], np.float32)
    adv_w = np.asarray(inputs["adv_w"], np.float32)
    v1w = np.asarray(inputs["v1w"], np.float32)
    adv_b = np.asarray(inputs["adv_b"], np.float32)
    v1b = np.asarray(inputs["v1b"], np.float32)

    src_i, tgt_i = ei[0], ei[1]
    wfull = np.concatenate([adv_w, v1w], axis=1)  # [32768, 256]

    hot = np.zeros((128, HOT_COLS), np.float32)
    hot[:, HOT_CB] = conv_b
    hot[0:64, HOT_W1] = w1.reshape(64)
    hot[0:64, HOT_B1] = b1

    pa0 = np.zeros((128, PA0_COLS), np.float32)
    pa0[:, PA_B2T : PA_B2T + F] = b2.reshape(F, OUT).T
    pa0[0:F, PA_X0T : PA_X0T + N] = x[0].T

    brow = np.concatenate([adv_b, v1b])          # [256]

    in_maps = []
    for c in range(M):
        hc = hot.copy()
        if c == 0:
            hc[:, HOT_BR0] = brow[0:128]
            hc[:, HOT_BR1] = brow[128:256]

        xb = np.zeros((F, XB_COLS), np.float32)
        xb[:, XRW : XRW + OUT] = root_w
        xb[:, XX0 : XX0 + NPC] = x[0, NPC * c : NPC * (c + 1), :].T
        # node stream: [f, node, batch-permuted]
        xc = x[PERM][:, NPC * c : NPC * (c + 1), :]     # [B, 32, 8]
        xb[:, XND:] = xc.transpose(2, 1, 0).reshape(F, NPC * B)

        pac = pa0.copy()
        # edge-count matrix for this core's 32 target nodes
        cmat = np.zeros((N, NPC), np.float32)
        sel = (tgt_i >= NPC * c) & (tgt_i < NPC * (c + 1))
        np.add.at(cmat, (src_i[sel], tgt_i[sel] - NPC * c), 1.0)
        pac[:, PA_C : PA_C + NPC] = cmat[0:128]
        pac[:, PA_C + NPC : PA_C + 2 * NPC] = cmat[128:256]

        rows = wfull[4096 * c : 4096 * (c + 1)]         # [4096, 256]
        wb = (
            rows.reshape(8, 4, 128, AV)
            .transpose(2, 0, 1, 3)
            .reshape(128, 8 * 1024)
        ).astype(ml_dtypes.bfloat16)
        in_maps.append(
            {"hot": hc, "xb": xb.astype(ml_dtypes.bfloat16),
             "wb": wb, "pa0": pac, "pw2": w2.astype(ml_dtypes.bfloat16)}
        )
    return in_maps


def _prep_phase2_inputs(inputs, pts):
    v2w = np.asarray(inputs["v2w"], np.float32)
    v2b = np.asarray(inputs["v2b"], np.float32)
    v3w = np.asarray(inputs["v3w"], np.float32)
    v3b = np.asarray(inputs["v3b"], np.float32)

    # structural dueling matrices (index structure only)
    dp = np.arange(NADV)
    mg = np.zeros((NADV, NDIV), np.float32)
    mg[dp, dp // PER] = -1.0 / PER           # negated group-mean matrix
    em = np.zeros((NDIV, NADV), np.float32)  # expand d -> (d,p)
    em[dp // PER, dp] = 1.0
    em0, em1 = em[:, :128], em[:, 128:]
    mga, mgb = mg[:128], mg[128:]

    import ml_dtypes

    hb2 = np.zeros((128, HB_COLS), np.float32)
    hb2[0:64, HB_V2B] = v2b
    hb2[:, HB_CB0] = v3b @ em0
    hb2[0:64, HB_CB1] = v3b @ em1

    pfm = np.zeros((128, PF_COLS), np.float32)
    pfm[64:128, PF_V2W : PF_V2W + 64] = v2w
    pfm[:, PF_A0 : PF_A0 + 128] = np.eye(128, dtype=np.float32) + mga @ em0
    pfm[0:64, PF_B0 : PF_B0 + 128] = mgb @ em0
    pfm[0:64, PF_C0 : PF_C0 + 128] = v3w @ em0
    pfm[:, PF_A1 : PF_A1 + 64] = mga @ em1
    pfm[0:64, PF_B1 : PF_B1 + 64] = np.eye(64, dtype=np.float32) + mgb @ em1
    pfm[0:64, PF_C1 : PF_C1 + 64] = v3w @ em1
    pfm = pfm.astype(ml_dtypes.bfloat16)

    in_maps = []
    for c in range(M):
        bsl = slice(BT * c, BT * (c + 1))
        stk = np.stack([p[:, bsl] for p in pts])          # [8, 256, BT]
        # [t, c', chunk, r, b] -> pp[32c'+r, h, t, k4, b]
        part2 = stk.reshape(2, 4, 8, 32, BT).transpose(1, 3, 0, 2, 4)
        pp = np.zeros((128, 2, 2, 4, BT), ml_dtypes.bfloat16)
        ppv = pp.reshape(4, 32, 2, 2, 4, BT)
        ppv[:, :, 0] = part2[:, :, :, 4:8]   # rows 128:256 (adv hi | v1)
        ppv[:, :, 1] = part2[:, :, :, 0:4]   # rows 0:128
        in_maps.append({"hb2": hb2, "pp": pp, "pf": pfm})
    return in_maps


class _Runner:
    """Cached PJRT executor for one Bass program across the 8 cores.

    Mirrors bass2jax.run_bass_via_pjrt but keeps the jitted callable so
    repeat calls don't re-trace/re-lower, enabling benchmarking.
    """

    def __init__(self, nc):
        import jax
        from jax.sharding import Mesh, PartitionSpec, NamedSharding
        from jax.experimental.shard_map import shard_map
        from concourse import bass2jax

        bass2jax.install_neuronx_cc_hook()
        self.jax = jax
        self.nc = nc
        partition_name = (
            nc.partition_id_tensor.name if nc.partition_id_tensor else None
        )
        in_names, out_names, out_avals, zero_shapes = [], [], [], []
        for alloc in nc.m.functions[0].allocations:
            if not isinstance(alloc, mybir.MemoryLocationSet):
                continue
            name = alloc.memorylocations[0].name
            if alloc.kind == "ExternalInput":
                if name != partition_name:
                    in_names.append(name)
            elif alloc.kind == "ExternalOutput":
                shape = tuple(alloc.tensor_shape)
                dtype = mybir.dt.np(alloc.dtype)
                out_names.append(name)
                out_avals.append(jax.core.ShapedArray(shape, dtype))
                zero_shapes.append((shape, dtype))
        self.in_names, self.out_names = in_names, out_names
        self.out_avals, self.zero_shapes = out_avals, zero_shapes
        n_params, n_outs = len(in_names), len(out_names)
        self.n_params = n_params

        bind_names = in_names + out_names
        if partition_name is not None:
            bind_names = bind_names + [partition_name]

        def _body(*args):
            operands = list(args)
            if partition_name is not None:
                operands.append(bass2jax.partition_id_tensor())
            outs = bass2jax._bass_exec_p.bind(
                *operands,
                out_avals=tuple(out_avals),
                in_names=tuple(bind_names),
                out_names=tuple(out_names),
                lowering_input_output_aliases=(),
                sim_require_finite=True,
                sim_require_nnan=True,
                nc=nc,
            )
            return tuple(outs)

        devices = jax.devices()[:M]
        self.mesh = Mesh(np.asarray(devices), ("core",))
        spec = PartitionSpec("core")
        self.sharding = NamedSharding(self.mesh, spec)
        donate = tuple(range(n_params, n_params + n_outs))
        self.fn = jax.jit(
            shard_map(
                _body,
                mesh=self.mesh,
                in_specs=(spec,) * (n_params + n_outs),
                out_specs=(spec,) * n_outs,
                check_rep=False,
            ),
            donate_argnums=donate,
            keep_unused=True,
        )

    def _concat_inputs(self, in_maps):
        return [
            np.concatenate([np.asarray(m[name]) for m in in_maps], axis=0)
            for name in self.in_names
        ]

    def _zeros(self):
        return [np.zeros((M * s[0], *s[1:]), d) for s, d in self.zero_shapes]

    def _split(self, out_arrs):
        res = []
        for c in range(M):
            res.append(
                {
                    name: np.asarray(out_arrs[i]).reshape(M, *self.out_avals[i].shape)[c]
                    for i, name in enumerate(self.out_names)
                }
            )
        return res

    def run(self, in_maps):
        out_arrs = self.fn(*self._concat_inputs(in_maps), *self._zeros())
        return self._split(out_arrs)

    def bench(self, in_maps, iters=20):
        import time

        jax = self.jax
        dev_in = [
            jax.device_put(a, self.sharding) for a in self._concat_inputs(in_maps)
        ]
        times = []
        out_arrs = None
        for _ in range(iters):
            zeros = [jax.device_put(z, self.sharding) for z in self._zeros()]
            jax.block_until_ready(zeros)
            t0 = time.perf_counter()
            out_arrs = self.fn(*dev_in, *zeros)
            jax.block_until_ready(out_arrs)
            times.append(time.perf_counter() - t0)
        return self._split(out_arrs), times


_runner_cache = {}


def _get_runner(nc, key):
    if key not in _runner_cache:
        _runner_cache[key] = _Runner(nc)
    return _runner_cache[key]


def _run_sim(nc, in_maps):
    from concourse.bass_interp import CoreSim

    outs = []
    for im in in_maps:
        sim = CoreSim(nc)
        for k, v in im.items():
            sim.tensor(k)[:] = v
        names = ("pt",) if "xb" in im else ("ot",)
        for n in names:
            sim.tensor(n)[:] = 0
        sim.simulate()
        outs.append({n: np.array(sim.tensor(n)) for n in names})
    return outs


def _run(inputs, mode=None, trace=False, backend="hw", bench_iters=0):
    nc1, nc2 = _get_programs(mode)
    info = {}

    in_maps1 = _prep_phase1_inputs(inputs, mode)
    if backend == "sim":
        res1 = _run_sim(nc1, in_maps1)
    else:
        runner1 = _get_runner(nc1, ("p1",))
        if bench_iters:
            res1, times = runner1.bench(in_maps1, bench_iters)
            info["phase1_ns"] = int(min(times) * 1e9)
            info["phase1_mean_ns"] = float(np.mean(times) * 1e9)
        else:
            res1 = runner1.run(in_maps1)
    import ml_dtypes
    pts = [
        np.asarray(res1[c]["pt"], ml_dtypes.bfloat16)
        .transpose(1, 0, 2).reshape(AV, B)
        for c in range(M)
    ]

    in_maps2 = _prep_phase2_inputs(inputs, pts)
    if backend == "sim":
        res2 = _run_sim(nc2, in_maps2)
    else:
        runner2 = _get_runner(nc2, ("p2",))
        if bench_iters:
            res2, times = runner2.bench(in_maps2, bench_iters)
            info["phase2_ns"] = int(min(times) * 1e9)
            info["phase2_mean_ns"] = float(np.mean(times) * 1e9)
        else:
            res2 = runner2.run(in_maps2)

    out = np.empty((B, NDIV, PER), np.float32)
    for c in range(M):
        ot = np.asarray(res2[c]["ot"], np.float32)  # [128, 2, BT]
        full = np.concatenate([ot[:, 0, :], ot[0:64, 1, :]], axis=0)
        out[BT * c : BT * (c + 1)] = full.T.reshape(BT, NDIV, PER)
    return out, info


def _p25(ts):
    ts = sorted(ts)
    return ts[max(0, len(ts) // 4)]


def bench_hw(inputs, mode=None, big_rep=9, iters=12):
    """Differential HW timing: (T(R) - T(1)) / (R - 1) cancels the axon
    launch overhead and measures the true per-pass device time.  Uses the
    25th percentile (the min is occasionally glitchy on the relay)."""
    in_maps1 = _prep_phase1_inputs(inputs, mode)
    res = {}
    est = {}
    for r in (1, big_rep):
        nc1, _ = _get_programs(mode, r)
        runner = _get_runner(nc1, ("p1", r))
        out1, times = runner.bench(in_maps1, iters)
        est[r] = _p25(times)
    res["phase1_ns"] = (est[big_rep] - est[1]) / (big_rep - 1) * 1e9
    res["phase1_launch_ns"] = est[1] * 1e9

    import ml_dtypes
    pts = [
        np.asarray(o["pt"], ml_dtypes.bfloat16)
        .transpose(1, 0, 2).reshape(AV, B)
        for o in out1
    ]
    in_maps2 = _prep_phase2_inputs(inputs, pts)
    for r in (1, big_rep):
        _, nc2 = _get_programs(mode, r)
        runner = _get_runner(nc2, ("p2", r))
        _, times = runner.bench(in_maps2, iters)
        est[r] = _p25(times)
    res["phase2_ns"] = (est[big_rep] - est[1]) / (big_rep - 1) * 1e9
    res["phase2_launch_ns"] = est[1] * 1e9
    return res


def kernel(**inputs):
    out, _ = _run(inputs)
    return out
